# revision 2
# baseline (speedup 1.0000x reference)
"""AxileAttention Trainium2 (deployed v4): fp16 + delta-PSUM + merged evacuation.

Per-core work (8 channels x 8 batches = 64 pairs): q = x@Wq + bq etc, then
softmax(q*k, axis=-1) * v, all per (batch, channel).

Key structure (vs kernel.py v1 baseline):
  * All inputs fp16 (near-one-hot softmax rows tolerate the logit error;
    validated ~8.5e-3 rel err vs the 2e-2 gate). Output fp16.
  * Bias preloads amortized once per channel via delta accumulation in
    persistent PSUM banks: kk parity-2 fed d2x_b = x_b - x_{b-2} (variant 0),
    q parity-2 fed the SAME d2x (shared variant!), v parity-4 fed
    d4x_b = x_b - x_{b-4} (variant 1). Deltas are host-computed against the
    fp32-reconstructed predecessor so fp16 rounding does not compound.
    PSUM = kk(2) + q(2) + v(4) = 8 banks exactly. PE ~3728 cyc/pair (~91%
    dense vs the ACT-bound ~1.7us pair period - keeps the HAM clock warm)
    vs 4674 for the per-pair-preload baseline.
  * kk lives in ONE [128, 2, 512] two-bank tile so the k evacuation runs as
    a single FD=1024 ScalarE copy per TWO pairs (one 224-cycle instruction
    overhead instead of two). q/v banks are separate tiles so the Tile
    framework tracks write-after-read per bank (a merged tile would
    serialize every batch on the previous chain read).
  * Chain per 2 pairs: merged kevac -> per pair/m: custom DVE ttr
    (s=-(q*k), rowmin accum), ACT exp (bias=-max, accum sums) -> one batched
    DVE fast-recip (FD=4) -> 4x DVE stt out=(p*(1/sum))*v straight from
    PSUM -> one 2-batch output DMA on the sync ring.
  * Loads: x chunked 2 batches/DMA on gpsimd SWDGE (per-partition data
    contiguous -> ~128 descriptors/DMA); weights as ONE [3,W,V] fp16 tensor
    and biases as ONE [3,H,V] fp16 tensor per channel on the scalar HWDGE
    ring (2 DMAs/channel instead of 6).
"""
import sys

sys.path.insert(0, "/opt/trn_rl_repo")

import numpy as np

import concourse.bacc as bacc
import concourse.tile as tile
import concourse.dve_ops as dve_ops
from concourse import mybir
from concourse.masks import make_identity
from concourse.dve_spec import C0, C1, Spec, Src0, Src1, lower, minn, _has_src1
from concourse.dve_uop import DveOpSpec

F32 = mybir.dt.float32
F16 = mybir.dt.float16

B = 8        # batch
C = 64       # channels total
CCH = 8      # channels per core
NCORES = 8
HP = 2       # h partition-tiles (h = 2j + m interleave)
KT = 2       # w partition-tiles (w = 2p + k interleave)
NV = 2       # x delta variants: 0 = d2 (kk+q), 1 = d4 (v)
H = W = V = 256
PAR_KK = 2
PAR_Q = 2
PAR_V = 4
XB = 2       # batches per x DMA


def _make_ttr_min():
    """Custom DVE op: out = (in0*in1)*s1 ; accum_out = min(s0, row-min of out).
    Called with s1=-1, s0=+BIG: out = -(q*k), accum = -rowmax(q*k)."""
    name = "TTR_MIN_NEG_ANT"
    for op in dve_ops.OPS:
        if op.name == name:
            return op
    spec = Spec(
        body=Src0 * Src1 * C1,
        accum=minn,
        accum_init=C0,
        reference=lambda in0, in1, s0, s1, imm2: (
            np.asarray(in0, np.float32) * in1 * s1,
            np.minimum(
                np.float32(s0),
                (np.asarray(in0, np.float32) * in1 * s1).min(-1, keepdims=True),
            ),
        ),
    )
    row = dve_ops._CUSTOM_DVE_ROW_BASE + len(dve_ops.OPS)
    assert row < 0x20
    shas = {
        ver: DveOpSpec(name=name, opcode=row, uops=lower(spec, ver=ver),
                       rd1_en=_has_src1(spec)).sha(ver)
        for ver in ("v3", "v4")
    }
    op = dve_ops.DveOp(name, spec, subdim=False, uops_sha=shas)
    dve_ops.OPS.append(op)
    dve_ops.CUSTOM_DVE_SPECS[name] = spec
    dve_ops._SUB_OPCODE_FOR_NAME[name] = row
    return op


def _build_nc(reps=1, xt_bufs=3, sb_bufs=3, out_bufs=4, wts_bufs=3,
              kevac_merge=True, act_warmup=True, wb_gpsimd=True,
              kevac_dve=0):
    ttr_min = _make_ttr_min()
    nc = bacc.Bacc("TRN2", target_bir_lowering=False, debug=False)
    xs = nc.dram_tensor("xs", [CCH, B // XB, 128, XB, KT, NV, H], F16,
                        kind="ExternalInput").ap()
    w3 = nc.dram_tensor("w3", [CCH, 3, W, V], F16, kind="ExternalInput").ap()
    b3 = nc.dram_tensor("b3", [CCH, 3, H, V], F16, kind="ExternalInput").ap()
    o = nc.dram_tensor("o", [B, CCH, H, V], F16, kind="ExternalOutput").ap()

    with tile.TileContext(nc) as tc:
        with (
            tc.tile_pool(name="const", bufs=1) as cpool,
            tc.tile_pool(name="wts", bufs=wts_bufs) as wpool,
            tc.tile_pool(name="sb", bufs=sb_bufs) as sb,
            tc.tile_pool(name="ps", bufs=1, space="PSUM") as ps,
        ):
            ident = cpool.tile([128, 128], F32)
            make_identity(nc, ident[:])
            ident_h = cpool.tile([128, 128], F16)
            nc.vector.tensor_copy(ident_h[:], ident[:])
            warmed = [False]

            def _act_warmup():
                if warmed[0] or not act_warmup:
                    return
                warmed[0] = True
                warm = cpool.tile([128, 1], F32)
                nc.scalar.activation(warm[:], ident_h[:, 0:1],
                                     mybir.ActivationFunctionType.Exp)

            def _body():
                # flat chunk pipeline: at chunk t emit [channel chores +
                # kk-data + kevac] for chunk t+1, then [q/v data + softmax
                # chain] for chunk t. The merged kevac (the chain head) then
                # runs a full chunk early, taking it off the critical path:
                # steady state is paced by per-engine busy time (~3.4us/chunk
                # on ACT), not the kevac->ttr->exp->stt serial latency.
                NCH = B // 2                        # chunks per channel
                flat = [(cc, ch) for cc in range(CCH) for ch in range(NCH)]
                chans = {}

                def _ensure_channel(cc):
                    if cc in chans:
                        return chans[cc]
                    w_mm = wpool.tile([128, 3, KT, V], F16, tag="w3", name="w3")
                    wb_eng = nc.gpsimd if wb_gpsimd else nc.sync
                    wb_eng.dma_start(
                        w_mm[:], w3[cc].rearrange("t (p k) v -> p t k v", k=KT))
                    b_mm = wpool.tile([128, 3, HP, V], F16, tag="b3", name="b3")
                    wb_eng.dma_start(
                        b_mm[:], b3[cc].rearrange("t (p m) v -> p t m v", m=HP))
                    _act_warmup()
                    st = dict(
                        w_mm=w_mm, b_mm=b_mm,
                        kk2=ps.tile([128, PAR_KK, 512], F32, tag="kk2", name="kk2"),
                        qpar=[ps.tile([128, 512], F32, tag=f"q{i}", name=f"q{i}")
                              for i in range(PAR_Q)],
                        vpar=[ps.tile([128, 512], F32, tag=f"v{i}", name=f"v{i}")
                              for i in range(PAR_V)],
                        xts={},
                    )
                    chans[cc] = st
                    return st

                def _load_xt(cc, g):
                    st = chans[cc]
                    if g not in st["xts"]:
                        t = sb.tile([128, XB, KT, NV, H], F16, tag="xT",
                                    bufs=xt_bufs, name="xT")
                        nc.gpsimd.dma_start(t[:], xs[cc, g])
                        st["xts"][g] = t
                    return st["xts"][g]

                def _mm(bank, xt, var, wt, kfull=False):
                    for m in range(HP):
                        for k in range(KT):
                            nc.tensor.matmul(
                                bank[:, m * 256:(m + 1) * 256],
                                xt[:, k, var, m * 128:(m + 1) * 128], wt[:, k],
                                start=False,
                                stop=(k == KT - 1 and (m == HP - 1 or not kfull)),
                                skip_group_check=True)

                def _kk_path(cc, ch):
                    # channel chores + kk preload/data + merged kevac for (cc, ch)
                    st = _ensure_channel(cc)
                    xt = _load_xt(cc, ch)
                    b_mm, w_mm, kk2 = st["b_mm"], st["w_mm"], st["kk2"]
                    if ch == 0:
                        bkf = b_mm[:, 1].rearrange("p m v -> p (m v)")
                        nc.tensor.matmul(kk2[:, 0], ident_h[:], bkf,
                                         start=True, stop=True)
                        nc.tensor.matmul(kk2[:, 1], ident_h[:], bkf,
                                         start=True, stop=True)
                    _mm(kk2[:, 0], xt[:, 0], 0, w_mm[:, 1], kfull=True)
                    _mm(kk2[:, 1], xt[:, 1], 0, w_mm[:, 1], kfull=True)
                    k_sb = sb.tile([128, PAR_KK, 512], F32, tag="ksb", name="ksb")
                    kf_dst = k_sb[:].rearrange("p t v -> p (t v)")
                    kf_src = kk2[:].rearrange("p t v -> p (t v)")
                    if kevac_dve:
                        # balance ACT vs DVE: tail of the merged evacuation
                        # rides the (slightly less busy) vector engine
                        cut = PAR_KK * 512 - kevac_dve
                        nc.scalar.copy(kf_dst[:, 0:cut], kf_src[:, 0:cut])
                        nc.vector.tensor_copy(kf_dst[:, cut:], kf_src[:, cut:])
                    else:
                        nc.scalar.copy(kf_dst, kf_src)
                    return k_sb

                def _qv_chain(cc, ch, k_sb):
                    st = chans[cc]
                    b0, b1 = 2 * ch, 2 * ch + 1
                    xt = st["xts"][ch]
                    w_mm, b_mm = st["w_mm"], st["b_mm"]
                    qpar, vpar = st["qpar"], st["vpar"]
                    if ch == 0:
                        bqf = b_mm[:, 0].rearrange("p m v -> p (m v)")
                        nc.tensor.matmul(qpar[0][:], ident_h[:], bqf,
                                         start=True, stop=True)
                        nc.tensor.matmul(qpar[1][:], ident_h[:], bqf,
                                         start=True, stop=True)
                    _mm(qpar[b0 % PAR_Q][:], xt[:, 0], 0, w_mm[:, 0])
                    _mm(qpar[b1 % PAR_Q][:], xt[:, 1], 0, w_mm[:, 0])
                    if b0 < PAR_V:
                        bvf = b_mm[:, 2].rearrange("p m v -> p (m v)")
                        nc.tensor.matmul(vpar[b0][:], ident_h[:], bvf,
                                         start=True, stop=True)
                        nc.tensor.matmul(vpar[b1][:], ident_h[:], bvf,
                                         start=True, stop=True)
                    _mm(vpar[b0 % PAR_V][:], xt[:, 0], 1, w_mm[:, 2])
                    _mm(vpar[b1 % PAR_V][:], xt[:, 1], 1, w_mm[:, 2])
                    # softmax chain for pairs (b0, b1)
                    s_sb = sb.tile([128, 2, HP, 256], F32, tag="s", name="s")
                    mneg = sb.tile([128, 2, HP], F32, tag="mneg", name="mneg")
                    sums = sb.tile([128, 2, HP], F32, tag="sums", name="sums")
                    p_sb = sb.tile([128, 2, HP, 256], F32, tag="p", name="p")
                    for i, b in enumerate((b0, b1)):
                        for m in range(HP):
                            nc.vector._custom_dve(
                                ttr_min,
                                out=s_sb[:, i, m],
                                in0=qpar[b % PAR_Q][:, m * 256:(m + 1) * 256],
                                in1=k_sb[:, b % PAR_KK, m * 256:(m + 1) * 256],
                                s0=3.0e38, s1=-1.0,
                                accum_out=mneg[:, i, m:m + 1],
                            )
                        for m in range(HP):
                            nc.scalar.activation(
                                p_sb[:, i, m], s_sb[:, i, m],
                                mybir.ActivationFunctionType.Exp,
                                bias=mneg[:, i, m:m + 1], scale=-1.0,
                                accum_out=sums[:, i, m:m + 1],
                            )
                    r_sb = sb.tile([128, 2, HP], F32, tag="r", name="r")
                    nc.vector.reciprocal_approx_fast(
                        r_sb[:].rearrange("p n m -> p (n m)"),
                        sums[:].rearrange("p n m -> p (n m)"))
                    out_sb = sb.tile([128, 2, HP, 256], F16, tag="out",
                                     bufs=out_bufs, name="out")
                    for i, b in enumerate((b0, b1)):
                        for m in range(HP):
                            nc.vector.scalar_tensor_tensor(
                                out_sb[:, i, m], p_sb[:, i, m],
                                r_sb[:, i, m:m + 1],
                                vpar[b % PAR_V][:, m * 256:(m + 1) * 256],
                                op0=mybir.AluOpType.mult,
                                op1=mybir.AluOpType.mult)
                    o_dst = o[b0:b1 + 1, cc].rearrange(
                        "b (p m) v -> p b m v", m=HP)
                    nc.sync.dma_start(o_dst, out_sb[:])

                ksb_cur = _kk_path(*flat[0])      # prologue
                for t, (cc, ch) in enumerate(flat):
                    ksb_nxt = _kk_path(*flat[t + 1]) if t + 1 < len(flat) else None
                    _qv_chain(cc, ch, ksb_cur)
                    ksb_cur = ksb_nxt

            if reps > 1:
                with tc.For_i(0, reps):
                    _body()
            else:
                _body()
    nc.compile()
    return nc


def _host_xpack(xc):
    """[B, CC, H, W] f32 -> [CC, B//XB, 128, XB, KT, NV, H] fp16.
    Partition p, tile k <-> w = 2p + k; h enumerated h' = m*128 + j <->
    h = 2j + m; variant 0 = d2 delta (kk+q), variant 1 = d4 delta (v).
    Deltas taken against the fp32-reconstructed predecessor (mirrors PSUM)."""
    B_, C_, H_, W_ = xc.shape
    xt = np.ascontiguousarray(xc.transpose(0, 1, 3, 2))  # [B, CC, W, H]
    xt = xt.reshape(B_, C_, W_, H_ // 2, 2).swapaxes(-1, -2).reshape(B_, C_, W_, H_)
    x16 = xt.astype(np.float16)

    def _chain(par):
        d = np.empty_like(x16)
        recon = {}
        for b in range(B_):
            p = b % par
            if p not in recon:
                d[b] = x16[b]
                recon[p] = x16[b].astype(np.float32)
            else:
                d[b] = (x16[b].astype(np.float32) - recon[p]).astype(np.float16)
                recon[p] = recon[p] + d[b].astype(np.float32)
        return d

    pack = np.stack([_chain(PAR_KK), _chain(PAR_V)], axis=3)  # [B,CC,W,NV,H]
    pack = pack.reshape(B_, C_, W_ // 2, 2, NV, H_)     # [B,CC,128,KT,NV,H]
    pack = pack.transpose(1, 0, 2, 3, 4, 5)             # [CC,B,128,KT,NV,H]
    pack = pack.reshape(C_, B_ // XB, XB, 128, KT, NV, H_).swapaxes(2, 3)
    return np.ascontiguousarray(pack)                   # [CC,B//XB,128,XB,KT,NV,H]


def shard_inputs(inputs):
    x = np.asarray(inputs["x"], np.float32)
    ws = [inputs["query_w"], inputs["key_w"], inputs["var_w"]]
    bs = [inputs["query_b"], inputs["key_b"], inputs["var_b"]]
    in_maps = []
    for c in range(NCORES):
        sl = slice(c * CCH, (c + 1) * CCH)
        in_maps.append({
            "xs": _host_xpack(x[:, sl]),
            "w3": np.ascontiguousarray(np.stack(
                [np.asarray(w)[sl].astype(np.float16) for w in ws], axis=1)),
            "b3": np.ascontiguousarray(np.stack(
                [np.asarray(b)[sl].astype(np.float16) for b in bs], axis=1)),
        })
    return in_maps


def kernel(x, query_w, key_w, var_w, query_b, key_b, var_b):
    from concourse.bass_utils import run_bass_kernel_spmd

    in_maps = shard_inputs(dict(x=x, query_w=query_w, key_w=key_w, var_w=var_w,
                                query_b=query_b, key_b=key_b, var_b=var_b))
    nc = _build_nc()
    res = run_bass_kernel_spmd(nc, in_maps, list(range(NCORES)))
    out = np.empty((B, C, H, V), np.float32)
    for c in range(NCORES):
        out[:, c * CCH:(c + 1) * CCH] = res.results[c]["o"]
    return out


# revision 3
# speedup vs baseline: 1.1312x; 1.1312x over previous
"""AxileAttention Trainium2 kernel (deployed v4): fp16 + delta-PSUM.

Problem: x[8,64,256,256] fp32; per-channel weights *_w[64,256,256], biases
*_b[64,256,256]:
    q = einsum("bchw,cwv->bchv", x, query_w) + query_b   (k, v likewise)
    out = softmax(q*k, axis=-1) * v
Sharding: channel axis C=64 -> 8 channels/core on 8 NeuronCores (weights
slice with the channels; batch rides along). Each core runs 64 (batch,
channel) pairs; outputs concatenate on the host.

Measured (delta-timed over an in-program 32x hardware loop, interleaved
with the previous kernel in the same noise window): ~104us vs ~116-160us
for the v1 baseline; timeline-sim total 125.7us with ScalarE saturated
(109.4us busy). Relative error 1.03e-2 (tolerance 2e-2).

How it gets there (vs the v1 per-pair-preload design, sim 134.8us):

* All device inputs fp16, output fp16. The softmax rows are near-one-hot
  (top-2 logit gaps ~100 >> the ~0.4 fp16-induced logit error), so fp16
  x/weights pass easily; only near-tie rows (P ~ 0.5%) even notice.
  Halves DMA (~30MB/core) and kills the f32r cast-DMAs and DVE wv-cast.
* Bias preloads amortized once per channel instead of 3 identity matmuls
  per pair: PSUM banks persist across the batch loop and the PE feeds
  DELTAS, d2x_b = x_b - x_{b-2} into kk and q parity banks (variant 0) and
  d4x_b = x_b - x_{b-4} into v parity banks (variant 1), so PSUM holds
  bias + x_b@W after every batch. Deltas are host-computed against the
  fp32-reconstructed predecessor (mirroring PSUM) so fp16 rounding does
  not compound. PSUM = kk(2) + q(2) + v(4) = 8 banks exactly.
  PE: 4674 -> ~3730 cyc/pair, ~91% duty vs the ACT-bound 1.7us pair
  period (dense enough to keep the HAM clock warm; a sparser PE stream
  measured ~2x slower on HW in one window - possibly throttle, possibly
  tunnel noise, but dense is also simply free here).
* kk occupies ONE [128, 2, 512] two-bank tile so the k evacuation is a
  single FD=1024 ScalarE copy per TWO pairs (one 224-cycle instruction
  overhead instead of two). q/v banks stay separate tiles so the Tile
  framework tracks write-after-read per bank. The sim's accumulation-group
  bookkeeping needs start=True,stop=True on the preloads (free on HW);
  the delta data matmuls all ride skip_group_check=True.
* Software pipeline: at flat-chunk t the PE stream carries [kk preload+
  data for chunk t+1], then [q, v data for chunk t]; the merged kevac for
  t+1 issues right behind its matmuls. The kevac therefore runs a full
  chunk early and the steady state is paced by per-engine busy time
  (~3.4us/chunk on ACT: 1 kevac + 4 exp + 4 accum-drains), not by the
  kevac->ttr->exp->stt serial latency. 4-pair kevac merging (kk par-4)
  simmed worse; per-pair chains with no merge simmed 114+ on ACT.
* Queue placement (each costs the issuing sequencer): x on gpsimd SWDGE,
  2 batches per DMA, host-packed so each partition is one contiguous 2KB
  run; weights as one [3,W,V] and biases as one [3,H,V] fp16 tensor per
  channel ALSO on gpsimd (on the scalar ring their issue bubbles cost the
  ACT engine ~24us of stalls; on sync they serialize behind output DMAs);
  outputs on the sync ring, one DMA per 2 pairs.
* Chain per 2 pairs (unchanged math): merged kevac; per pair/m a custom
  DVE op TTR_MIN_NEG_ANT computes s=-(q*k) and -rowmax in one pass (the
  native tensor_tensor_reduce encodes but fails on HW - keep the custom
  op); ACT exp with per-partition bias and accumulated row sums; one
  batched DVE fast reciprocal (FD=4); DVE scalar_tensor_tensor
  out=(p*(1/sum))*v reading v straight from PSUM, writing fp16.
* Engine budget per core, timeline-sim: ACT 109.4us busy (the wall:
  64 pairs x (520 kevac + 800 exp + 374 accum-drain reads)), DVE 102.6
  (ttr + stt + recip), PE 102-116 (matmuls), DMA ~87, gpsimd ~49,
  HWDGE ring ~20. Exp-table warmup rides the first channel's loads.
"""
import sys

sys.path.insert(0, "/opt/trn_rl_repo")

import numpy as np

import concourse.bacc as bacc
import concourse.tile as tile
import concourse.dve_ops as dve_ops
from concourse import mybir
from concourse.masks import make_identity
from concourse.dve_spec import C0, C1, Spec, Src0, Src1, lower, minn, _has_src1
from concourse.dve_uop import DveOpSpec

F32 = mybir.dt.float32
F16 = mybir.dt.float16

B = 8        # batch
C = 64       # channels total
CCH = 8      # channels per core
NCORES = 8
HP = 2       # h partition-tiles (h = 2j + m interleave)
KT = 2       # w partition-tiles (w = 2p + k interleave)
NV = 2       # x delta variants: 0 = d2 (kk+q), 1 = d4 (v)
H = W = V = 256
PAR_KK = 2
PAR_Q = 2
PAR_V = 4
XB = 2       # batches per x DMA


def _make_ttr_min():
    """Custom DVE op: out = (in0*in1)*s1 ; accum_out = min(s0, row-min of out).
    Called with s1=-1, s0=+BIG: out = -(q*k), accum = -rowmax(q*k)."""
    name = "TTR_MIN_NEG_ANT"
    for op in dve_ops.OPS:
        if op.name == name:
            return op
    spec = Spec(
        body=Src0 * Src1 * C1,
        accum=minn,
        accum_init=C0,
        reference=lambda in0, in1, s0, s1, imm2: (
            np.asarray(in0, np.float32) * in1 * s1,
            np.minimum(
                np.float32(s0),
                (np.asarray(in0, np.float32) * in1 * s1).min(-1, keepdims=True),
            ),
        ),
    )
    row = dve_ops._CUSTOM_DVE_ROW_BASE + len(dve_ops.OPS)
    assert row < 0x20
    shas = {
        ver: DveOpSpec(name=name, opcode=row, uops=lower(spec, ver=ver),
                       rd1_en=_has_src1(spec)).sha(ver)
        for ver in ("v3", "v4")
    }
    op = dve_ops.DveOp(name, spec, subdim=False, uops_sha=shas)
    dve_ops.OPS.append(op)
    dve_ops.CUSTOM_DVE_SPECS[name] = spec
    dve_ops._SUB_OPCODE_FOR_NAME[name] = row
    return op


def _build_nc(reps=1, xt_bufs=3, sb_bufs=3, out_bufs=4, wts_bufs=3,
              kevac_merge=True, act_warmup=True, wb_gpsimd=True,
              kevac_dve=0):
    ttr_min = _make_ttr_min()
    nc = bacc.Bacc("TRN2", target_bir_lowering=False, debug=False)
    xs = nc.dram_tensor("xs", [CCH, B // XB, 128, XB, KT, NV, H], F16,
                        kind="ExternalInput").ap()
    w3 = nc.dram_tensor("w3", [CCH, 3, W, V], F16, kind="ExternalInput").ap()
    b3 = nc.dram_tensor("b3", [CCH, 3, H, V], F16, kind="ExternalInput").ap()
    o = nc.dram_tensor("o", [B, CCH, H, V], F16, kind="ExternalOutput").ap()

    with tile.TileContext(nc) as tc:
        with (
            tc.tile_pool(name="const", bufs=1) as cpool,
            tc.tile_pool(name="wts", bufs=wts_bufs) as wpool,
            tc.tile_pool(name="sb", bufs=sb_bufs) as sb,
            tc.tile_pool(name="ps", bufs=1, space="PSUM") as ps,
        ):
            ident = cpool.tile([128, 128], F32)
            make_identity(nc, ident[:])
            ident_h = cpool.tile([128, 128], F16)
            nc.vector.tensor_copy(ident_h[:], ident[:])
            warmed = [False]

            def _act_warmup():
                if warmed[0] or not act_warmup:
                    return
                warmed[0] = True
                warm = cpool.tile([128, 1], F32)
                nc.scalar.activation(warm[:], ident_h[:, 0:1],
                                     mybir.ActivationFunctionType.Exp)

            def _body():
                # flat chunk pipeline: at chunk t emit [channel chores +
                # kk-data + kevac] for chunk t+1, then [q/v data + softmax
                # chain] for chunk t. The merged kevac (the chain head) then
                # runs a full chunk early, taking it off the critical path:
                # steady state is paced by per-engine busy time (~3.4us/chunk
                # on ACT), not the kevac->ttr->exp->stt serial latency.
                NCH = B // 2                        # chunks per channel
                flat = [(cc, ch) for cc in range(CCH) for ch in range(NCH)]
                chans = {}

                def _ensure_channel(cc):
                    if cc in chans:
                        return chans[cc]
                    w_mm = wpool.tile([128, 3, KT, V], F16, tag="w3", name="w3")
                    wb_eng = nc.gpsimd if wb_gpsimd else nc.sync
                    wb_eng.dma_start(
                        w_mm[:], w3[cc].rearrange("t (p k) v -> p t k v", k=KT))
                    b_mm = wpool.tile([128, 3, HP, V], F16, tag="b3", name="b3")
                    wb_eng.dma_start(
                        b_mm[:], b3[cc].rearrange("t (p m) v -> p t m v", m=HP))
                    _act_warmup()
                    st = dict(
                        w_mm=w_mm, b_mm=b_mm,
                        kk2=ps.tile([128, PAR_KK, 512], F32, tag="kk2", name="kk2"),
                        qpar=[ps.tile([128, 512], F32, tag=f"q{i}", name=f"q{i}")
                              for i in range(PAR_Q)],
                        vpar=[ps.tile([128, 512], F32, tag=f"v{i}", name=f"v{i}")
                              for i in range(PAR_V)],
                        xts={},
                    )
                    chans[cc] = st
                    return st

                def _load_xt(cc, g):
                    st = chans[cc]
                    if g not in st["xts"]:
                        t = sb.tile([128, XB, KT, NV, H], F16, tag="xT",
                                    bufs=xt_bufs, name="xT")
                        nc.gpsimd.dma_start(t[:], xs[cc, g])
                        st["xts"][g] = t
                    return st["xts"][g]

                def _mm(bank, xt, var, wt, kfull=False):
                    for m in range(HP):
                        for k in range(KT):
                            nc.tensor.matmul(
                                bank[:, m * 256:(m + 1) * 256],
                                xt[:, k, var, m * 128:(m + 1) * 128], wt[:, k],
                                start=False,
                                stop=(k == KT - 1 and (m == HP - 1 or not kfull)),
                                skip_group_check=True)

                def _kk_path(cc, ch):
                    # channel chores + kk preload/data + merged kevac for (cc, ch)
                    st = _ensure_channel(cc)
                    xt = _load_xt(cc, ch)
                    b_mm, w_mm, kk2 = st["b_mm"], st["w_mm"], st["kk2"]
                    if ch == 0:
                        bkf = b_mm[:, 1].rearrange("p m v -> p (m v)")
                        nc.tensor.matmul(kk2[:, 0], ident_h[:], bkf,
                                         start=True, stop=True)
                        nc.tensor.matmul(kk2[:, 1], ident_h[:], bkf,
                                         start=True, stop=True)
                    _mm(kk2[:, 0], xt[:, 0], 0, w_mm[:, 1], kfull=True)
                    _mm(kk2[:, 1], xt[:, 1], 0, w_mm[:, 1], kfull=True)
                    k_sb = sb.tile([128, PAR_KK, 512], F32, tag="ksb", name="ksb")
                    kf_dst = k_sb[:].rearrange("p t v -> p (t v)")
                    kf_src = kk2[:].rearrange("p t v -> p (t v)")
                    if kevac_dve:
                        # balance ACT vs DVE: tail of the merged evacuation
                        # rides the (slightly less busy) vector engine
                        cut = PAR_KK * 512 - kevac_dve
                        nc.scalar.copy(kf_dst[:, 0:cut], kf_src[:, 0:cut])
                        nc.vector.tensor_copy(kf_dst[:, cut:], kf_src[:, cut:])
                    else:
                        nc.scalar.copy(kf_dst, kf_src)
                    return k_sb

                def _qv_chain(cc, ch, k_sb):
                    st = chans[cc]
                    b0, b1 = 2 * ch, 2 * ch + 1
                    xt = st["xts"][ch]
                    w_mm, b_mm = st["w_mm"], st["b_mm"]
                    qpar, vpar = st["qpar"], st["vpar"]
                    if ch == 0:
                        bqf = b_mm[:, 0].rearrange("p m v -> p (m v)")
                        nc.tensor.matmul(qpar[0][:], ident_h[:], bqf,
                                         start=True, stop=True)
                        nc.tensor.matmul(qpar[1][:], ident_h[:], bqf,
                                         start=True, stop=True)
                    _mm(qpar[b0 % PAR_Q][:], xt[:, 0], 0, w_mm[:, 0])
                    _mm(qpar[b1 % PAR_Q][:], xt[:, 1], 0, w_mm[:, 0])
                    if b0 < PAR_V:
                        bvf = b_mm[:, 2].rearrange("p m v -> p (m v)")
                        nc.tensor.matmul(vpar[b0][:], ident_h[:], bvf,
                                         start=True, stop=True)
                        nc.tensor.matmul(vpar[b1][:], ident_h[:], bvf,
                                         start=True, stop=True)
                    _mm(vpar[b0 % PAR_V][:], xt[:, 0], 1, w_mm[:, 2])
                    _mm(vpar[b1 % PAR_V][:], xt[:, 1], 1, w_mm[:, 2])
                    # softmax chain for pairs (b0, b1)
                    s_sb = sb.tile([128, 2, HP, 256], F32, tag="s", name="s")
                    mneg = sb.tile([128, 2, HP], F32, tag="mneg", name="mneg")
                    sums = sb.tile([128, 2, HP], F32, tag="sums", name="sums")
                    p_sb = sb.tile([128, 2, HP, 256], F32, tag="p", name="p")
                    for i, b in enumerate((b0, b1)):
                        for m in range(HP):
                            nc.vector._custom_dve(
                                ttr_min,
                                out=s_sb[:, i, m],
                                in0=qpar[b % PAR_Q][:, m * 256:(m + 1) * 256],
                                in1=k_sb[:, b % PAR_KK, m * 256:(m + 1) * 256],
                                s0=3.0e38, s1=-1.0,
                                accum_out=mneg[:, i, m:m + 1],
                            )
                        for m in range(HP):
                            nc.scalar.activation(
                                p_sb[:, i, m], s_sb[:, i, m],
                                mybir.ActivationFunctionType.Exp,
                                bias=mneg[:, i, m:m + 1], scale=-1.0,
                                accum_out=sums[:, i, m:m + 1],
                            )
                    r_sb = sb.tile([128, 2, HP], F32, tag="r", name="r")
                    nc.vector.reciprocal_approx_fast(
                        r_sb[:].rearrange("p n m -> p (n m)"),
                        sums[:].rearrange("p n m -> p (n m)"))
                    out_sb = sb.tile([128, 2, HP, 256], F16, tag="out",
                                     bufs=out_bufs, name="out")
                    for i, b in enumerate((b0, b1)):
                        for m in range(HP):
                            nc.vector.scalar_tensor_tensor(
                                out_sb[:, i, m], p_sb[:, i, m],
                                r_sb[:, i, m:m + 1],
                                vpar[b % PAR_V][:, m * 256:(m + 1) * 256],
                                op0=mybir.AluOpType.mult,
                                op1=mybir.AluOpType.mult)
                    o_dst = o[b0:b1 + 1, cc].rearrange(
                        "b (p m) v -> p b m v", m=HP)
                    nc.sync.dma_start(o_dst, out_sb[:])

                ksb_cur = _kk_path(*flat[0])      # prologue
                for t, (cc, ch) in enumerate(flat):
                    ksb_nxt = _kk_path(*flat[t + 1]) if t + 1 < len(flat) else None
                    _qv_chain(cc, ch, ksb_cur)
                    ksb_cur = ksb_nxt

            if reps > 1:
                with tc.For_i(0, reps):
                    _body()
            else:
                _body()
    nc.compile()
    return nc


def _host_xpack(xc):
    """[B, CC, H, W] f32 -> [CC, B//XB, 128, XB, KT, NV, H] fp16.
    Partition p, tile k <-> w = 2p + k; h enumerated h' = m*128 + j <->
    h = 2j + m; variant 0 = d2 delta (kk+q), variant 1 = d4 delta (v).
    Deltas taken against the fp32-reconstructed predecessor (mirrors PSUM)."""
    B_, C_, H_, W_ = xc.shape
    xt = np.ascontiguousarray(xc.transpose(0, 1, 3, 2))  # [B, CC, W, H]
    xt = xt.reshape(B_, C_, W_, H_ // 2, 2).swapaxes(-1, -2).reshape(B_, C_, W_, H_)
    x16 = xt.astype(np.float16)

    def _chain(par):
        d = np.empty_like(x16)
        recon = {}
        for b in range(B_):
            p = b % par
            if p not in recon:
                d[b] = x16[b]
                recon[p] = x16[b].astype(np.float32)
            else:
                d[b] = (x16[b].astype(np.float32) - recon[p]).astype(np.float16)
                recon[p] = recon[p] + d[b].astype(np.float32)
        return d

    pack = np.stack([_chain(PAR_KK), _chain(PAR_V)], axis=3)  # [B,CC,W,NV,H]
    pack = pack.reshape(B_, C_, W_ // 2, 2, NV, H_)     # [B,CC,128,KT,NV,H]
    pack = pack.transpose(1, 0, 2, 3, 4, 5)             # [CC,B,128,KT,NV,H]
    pack = pack.reshape(C_, B_ // XB, XB, 128, KT, NV, H_).swapaxes(2, 3)
    return np.ascontiguousarray(pack)                   # [CC,B//XB,128,XB,KT,NV,H]


def shard_inputs(inputs):
    x = np.asarray(inputs["x"], np.float32)
    ws = [inputs["query_w"], inputs["key_w"], inputs["var_w"]]
    bs = [inputs["query_b"], inputs["key_b"], inputs["var_b"]]
    in_maps = []
    for c in range(NCORES):
        sl = slice(c * CCH, (c + 1) * CCH)
        in_maps.append({
            "xs": _host_xpack(x[:, sl]),
            "w3": np.ascontiguousarray(np.stack(
                [np.asarray(w)[sl].astype(np.float16) for w in ws], axis=1)),
            "b3": np.ascontiguousarray(np.stack(
                [np.asarray(b)[sl].astype(np.float16) for b in bs], axis=1)),
        })
    return in_maps


def kernel(x, query_w, key_w, var_w, query_b, key_b, var_b):
    from concourse.bass_utils import run_bass_kernel_spmd

    in_maps = shard_inputs(dict(x=x, query_w=query_w, key_w=key_w, var_w=var_w,
                                query_b=query_b, key_b=key_b, var_b=var_b))
    nc = _build_nc()
    res = run_bass_kernel_spmd(nc, in_maps, list(range(NCORES)))
    out = np.empty((B, C, H, V), np.float32)
    for c in range(NCORES):
        out[:, c * CCH:(c + 1) * CCH] = res.results[c]["o"]
    return out
